# revision 8
# baseline (speedup 1.0000x reference)
"""GATv2 attention head (kgcnn AttentionHeadGATV2) on 8 Trainium2 NeuronCores.

Strategy (edge/graph parallelism, self-contained — no file reads):
  * Host: bucket edges by receiver node; core c owns receiver nodes
    [c*6250, (c+1)*6250). Within a core, edges are grouped into 49
    receiver "windows" of 128 nodes, each padded to a fixed capacity
    (split into send<32768 / send>=32768 halves so gather indices fit
    int16). All index/metadata arrays are precomputed on host.
  * Device precompute (replicated per core): psw = [P_s | w_n] where
    P_s = node @ (W_att[128:]*c)[:,perm],  w_n = node @ W_lin + b_lin,
    and a per-core shard of P_r = node @ (W_att[:128]*c)[:,perm] + b_hat.
    The column scaling c and permutation fold a_vec into the leaky-relu
    (sign trick), so the attention logit becomes a plain row-sum.
  * Device edge phase, per window: dma_gather P_r[recv], [P_s|w_n][send];
    z = pr + ps; h = lrelu_{0.2}/lrelu_{5}(z) (column split at k);
    a = rowsum(h); ex = exp(a) via tanh identity; one-hot (x ex) built by
    fused tensor_scalar(is_equal, mult); scatter-add via PE matmul into a
    PSUM window [128 nodes, 128+1] accumulating numerator and denominator;
    out = lrelu_{0.2}(numer/denom).
"""

import sys

sys.path.insert(0, "/opt/trn_rl_repo")

import numpy as np
import ml_dtypes

import concourse.bacc as bacc
import concourse.bass as bass
import concourse.mybir as mybir
import concourse.tile as tile
from concourse import bass_utils

DT = mybir.dt
ALU = mybir.AluOpType
ACTF = mybir.ActivationFunctionType
AXIS = mybir.AxisListType

BF16 = ml_dtypes.bfloat16

# Problem constants (hardcoded per the task contract).
N_NODES = 50000
N_EDGES = 800000
F_DIM = 128
UNITS = 128
ALPHA = 0.2
NCORES = 8
NPC = N_NODES // NCORES          # 6250 receiver nodes per core
WIN = 128                        # receiver-window size (PSUM partition dim)
NW = (NPC + WIN - 1) // WIN      # 49 windows per core
LAST_ROWS = NPC - (NW - 1) * WIN # 106 rows in the last window
NODE_PAD = 50176                 # 98 * 512 (dma-transpose supertiles)
SHARD_PAD = 6656                 # 13 * 512
LO_BASE = 32768                  # int16 gather-index split point
ST_ROWS = 512                    # precompute supertile rows

_BUILD_CACHE = {}

DEFAULT_CFG = dict(
    n_nodes=N_NODES, n_edges=N_EDGES, ncores=NCORES, npc=NPC, nw=NW,
    last_rows=LAST_ROWS, node_pad=NODE_PAD, shard_pad=SHARD_PAD,
    lo_base=LO_BASE,
)


def _build(cap_lo, cap_hi, k_pos, cfg=None, compile=True, dbg=False):
    """Build + compile the SPMD Bass program.

    cap_lo/cap_hi: per-window fixed slot capacities (multiples of 128).
    k_pos: number of features with a_vec >= 0 (column split for lrelu).
    """
    p = dict(DEFAULT_CFG)
    if cfg is not None:
        p.update(cfg)

    nsub_lo = cap_lo // 128
    nsub_hi = cap_hi // 128
    cap = cap_lo + cap_hi
    nsub = nsub_lo + nsub_hi
    cap16 = cap // 16
    lo16 = cap_lo // 16
    hi16 = cap_hi // 16
    nw = p["nw"]

    nc = bacc.Bacc("TRN2", target_bir_lowering=False, debug=False,
                   num_devices=p["ncores"])

    node_pad_d = nc.dram_tensor("node_pad", [p["node_pad"], 128], DT.bfloat16,
                                kind="ExternalInput")
    node_shard_d = nc.dram_tensor("node_shard", [p["shard_pad"], 128],
                                  DT.bfloat16, kind="ExternalInput")
    w_s_d = nc.dram_tensor("w_s", [128, 128], DT.bfloat16, kind="ExternalInput")
    w_lin_d = nc.dram_tensor("w_lin", [128, 128], DT.bfloat16,
                             kind="ExternalInput")
    w_r_d = nc.dram_tensor("w_r", [128, 128], DT.bfloat16, kind="ExternalInput")
    bias_pair_d = nc.dram_tensor("bias_pair", [128, 256], DT.float32,
                                 kind="ExternalInput")
    bias_r_d = nc.dram_tensor("bias_r", [128, 128], DT.float32,
                              kind="ExternalInput")
    iota_d = nc.dram_tensor("iota", [128, 128], DT.float32,
                            kind="ExternalInput")
    recvidx_d = nc.dram_tensor("recvidx", [128, nw * cap16], DT.int16,
                               kind="ExternalInput")
    sendlo_d = nc.dram_tensor("sendlo", [128, nw * lo16], DT.int16,
                              kind="ExternalInput")
    sendhi_d = nc.dram_tensor("sendhi", [128, nw * hi16], DT.int16,
                              kind="ExternalInput")
    recvadj_d = nc.dram_tensor("recvadj", [128, nw * nsub], DT.float32,
                               kind="ExternalInput")
    out_d = nc.dram_tensor("out", [p["npc"], 128], DT.float32,
                           kind="ExternalOutput")
    if dbg:
        nsub_t = (cap_lo + cap_hi) // 128
        dbg_psw = nc.dram_tensor("dbg_psw", [p["node_pad"], 256], DT.float32,
                                 kind="ExternalOutput")
        dbg_pr = nc.dram_tensor("dbg_pr", [p["shard_pad"], 128], DT.float32,
                                kind="ExternalOutput")
        dbg_gpr = nc.dram_tensor("dbg_gpr", [128, nsub_t, 128], DT.float32,
                                 kind="ExternalOutput")
        dbg_gpsw = nc.dram_tensor("dbg_gpsw", [128, nsub_t, 256], DT.float32,
                                  kind="ExternalOutput")
        dbg_z = nc.dram_tensor("dbg_z", [128, nsub_t, 128], DT.float32,
                               kind="ExternalOutput")
        dbg_h = nc.dram_tensor("dbg_h", [128, nsub_t, 128], DT.float32,
                               kind="ExternalOutput")
        dbg_a = nc.dram_tensor("dbg_a", [128, nsub_t], DT.float32,
                               kind="ExternalOutput")
        dbg_ex = nc.dram_tensor("dbg_ex", [128, nsub_t], DT.float32,
                                kind="ExternalOutput")
        dbg_oh = nc.dram_tensor("dbg_oh", [128, nsub_t, 128], DT.float32,
                                kind="ExternalOutput")
        dbg_pn = nc.dram_tensor("dbg_pn", [128, 128], DT.float32,
                                kind="ExternalOutput")
        dbg_pd = nc.dram_tensor("dbg_pd", [128, 1], DT.float32,
                                kind="ExternalOutput")

    with tile.TileContext(nc) as tc:
        with (
            tc.tile_pool(name="consts", bufs=1) as cpool,
            tc.tile_pool(name="dram", bufs=1, space="DRAM") as dpool,
        ):
            psw_dram = dpool.tile([p["node_pad"], 256], DT.float32)
            pr_dram = dpool.tile([p["shard_pad"], 128], DT.float32)

            ws_sb = cpool.tile([128, 128], DT.bfloat16)
            nc.sync.dma_start(ws_sb[:], w_s_d[:])
            wlin_sb = cpool.tile([128, 128], DT.bfloat16)
            nc.sync.dma_start(wlin_sb[:], w_lin_d[:])
            wr_sb = cpool.tile([128, 128], DT.bfloat16)
            nc.sync.dma_start(wr_sb[:], w_r_d[:])
            biasp_sb = cpool.tile([128, 256], DT.float32)
            nc.sync.dma_start(biasp_sb[:], bias_pair_d[:])
            biasr_sb = cpool.tile([128, 128], DT.float32)
            nc.sync.dma_start(biasr_sb[:], bias_r_d[:])
            iota_sb = cpool.tile([128, 128], DT.float32)
            nc.sync.dma_start(iota_sb[:], iota_d[:])
            recvidx_sb = cpool.tile([128, nw * cap16], DT.int16)
            nc.sync.dma_start(recvidx_sb[:], recvidx_d[:])
            sendlo_sb = cpool.tile([128, nw * lo16], DT.int16)
            nc.sync.dma_start(sendlo_sb[:], sendlo_d[:])
            sendhi_sb = cpool.tile([128, nw * hi16], DT.int16)
            nc.sync.dma_start(sendhi_sb[:], sendhi_d[:])
            recvadj_sb = cpool.tile([128, nw * nsub], DT.float32)
            nc.sync.dma_start(recvadj_sb[:], recvadj_d[:])
            ones_sb = cpool.tile([128, 1], DT.float32)
            nc.vector.memset(ones_sb[:], 1.0)

            # ---------------- precompute phase ----------------
            with (
                tc.tile_pool(name="pcsb", bufs=3) as pc,
                tc.tile_pool(name="pcpsum", bufs=4, space="PSUM") as pcp,
            ):
                for st in range(p["node_pad"] // ST_ROWS):
                    ntile = pc.tile([128, ST_ROWS], DT.bfloat16, tag="nodeT")
                    nc.sync.dma_start(
                        ntile[:], node_pad_d[st * ST_ROWS:(st + 1) * ST_ROWS, :],
                        transpose=True)
                    for j in range(ST_ROWS // 128):
                        ps = pcp.tile([128, 256], DT.float32, tag="pcps")
                        lhsT = ntile[:, j * 128:(j + 1) * 128]
                        nc.tensor.matmul(ps[:, 0:128], lhsT, ws_sb[:],
                                         start=True, stop=True)
                        nc.tensor.matmul(ps[:, 128:256], lhsT, wlin_sb[:],
                                         start=True, stop=True)
                        row = pc.tile([128, 256], DT.float32, tag="pswrow")
                        nc.scalar.copy(row[:, 0:128], ps[:, 0:128])
                        nc.vector.tensor_tensor(row[:, 128:256], ps[:, 128:256],
                                                biasp_sb[:, 128:256], ALU.add)
                        r0 = st * ST_ROWS + j * 128
                        nc.sync.dma_start(psw_dram[r0:r0 + 128, :], row[:])
                for st in range(p["shard_pad"] // ST_ROWS):
                    ntile = pc.tile([128, ST_ROWS], DT.bfloat16, tag="nodeT")
                    nc.sync.dma_start(
                        ntile[:],
                        node_shard_d[st * ST_ROWS:(st + 1) * ST_ROWS, :],
                        transpose=True)
                    for j in range(ST_ROWS // 128):
                        ps = pcp.tile([128, 256], DT.float32, tag="pcps")
                        lhsT = ntile[:, j * 128:(j + 1) * 128]
                        nc.tensor.matmul(ps[:, 0:128], lhsT, wr_sb[:],
                                         start=True, stop=True)
                        row = pc.tile([128, 256], DT.float32, tag="pswrow")
                        nc.vector.tensor_tensor(row[:, 0:128], ps[:, 0:128],
                                                biasr_sb[:], ALU.add)
                        r0 = st * ST_ROWS + j * 128
                        nc.sync.dma_start(pr_dram[r0:r0 + 128, :],
                                          row[:, 0:128])

            if dbg:
                nc.sync.dma_start(dbg_psw[:], psw_dram[:])
                nc.sync.dma_start(dbg_pr[:], pr_dram[:])

            # ---------------- edge phase ----------------
            with (
                tc.tile_pool(name="edge", bufs=2) as ep,
                tc.tile_pool(name="small", bufs=2) as sp,
                tc.tile_pool(name="epsum", bufs=2, space="PSUM") as pp,
            ):
                for w in range(nw):
                    GMAX = 1024
                    gpr = ep.tile([128, nsub, 128], DT.float32, tag="gpr")
                    for g0 in range(0, cap, GMAX):
                        gn = min(GMAX, cap - g0)
                        nc.gpsimd.dma_gather(
                            gpr[:, g0 // 128:(g0 + gn) // 128, :], pr_dram[:],
                            recvidx_sb[:, w * cap16 + g0 // 16:
                                       w * cap16 + (g0 + gn) // 16],
                            gn, gn, 128, queue_num=0)
                    gpsw = ep.tile([128, nsub, 256], DT.float32, tag="gpsw")
                    for g0 in range(0, cap_lo, GMAX):
                        gn = min(GMAX, cap_lo - g0)
                        nc.gpsimd.dma_gather(
                            gpsw[:, g0 // 128:(g0 + gn) // 128, :], psw_dram[:],
                            sendlo_sb[:, w * lo16 + g0 // 16:
                                      w * lo16 + (g0 + gn) // 16],
                            gn, gn, 256, queue_num=0)
                    for g0 in range(0, cap_hi, GMAX):
                        gn = min(GMAX, cap_hi - g0)
                        nc.gpsimd.dma_gather(
                            gpsw[:, nsub_lo + g0 // 128:
                                 nsub_lo + (g0 + gn) // 128, :],
                            psw_dram[p["lo_base"]:, :],
                            sendhi_sb[:, w * hi16 + g0 // 16:
                                      w * hi16 + (g0 + gn) // 16],
                            gn, gn, 256, queue_num=0)

                    z = ep.tile([128, nsub, 128], DT.float32, tag="z")
                    nc.vector.tensor_tensor(z[:], gpr[:], gpsw[:, :, 0:128],
                                            ALU.add)
                    h = ep.tile([128, nsub, 128], DT.float32, tag="h")
                    if k_pos > 0:
                        nc.vector.scalar_tensor_tensor(
                            h[:, :, 0:k_pos], z[:, :, 0:k_pos], ALPHA,
                            z[:, :, 0:k_pos], ALU.mult, ALU.max)
                    if k_pos < 128:
                        nc.vector.scalar_tensor_tensor(
                            h[:, :, k_pos:128], z[:, :, k_pos:128], 1.0 / ALPHA,
                            z[:, :, k_pos:128], ALU.mult, ALU.min)
                    a = sp.tile([128, nsub], DT.float32, tag="a")
                    nc.vector.tensor_reduce(a[:], h[:], AXIS.X, ALU.add)
                    # exp(a) = (1 + tanh(a/2)) / (1 - tanh(a/2))
                    t = sp.tile([128, nsub], DT.float32, tag="t")
                    nc.scalar.activation(t[:], a[:], ACTF.Tanh, scale=0.5)
                    d = sp.tile([128, nsub], DT.float32, tag="d")
                    nc.vector.tensor_scalar(d[:], t[:], -1.0, 1.0, ALU.mult,
                                            ALU.add)
                    rd = sp.tile([128, nsub], DT.float32, tag="rd")
                    nc.vector.reciprocal(rd[:], d[:])
                    ex = sp.tile([128, nsub], DT.float32, tag="ex")
                    nc.vector.scalar_tensor_tensor(ex[:], t[:], 1.0, rd[:],
                                                   ALU.add, ALU.mult)

                    oh = ep.tile([128, nsub, 128], DT.float32, tag="oh")
                    for s in range(nsub):
                        nc.vector.tensor_scalar(
                            oh[:, s, :], iota_sb[:],
                            recvadj_sb[:, w * nsub + s:w * nsub + s + 1],
                            ex[:, s:s + 1], ALU.is_equal, ALU.mult)

                    pw_n = pp.tile([128, 128], DT.float32, tag="pwn")
                    pw_d = pp.tile([128, 1], DT.float32, tag="pwd")
                    for s in range(nsub):
                        nc.tensor.matmul(pw_n[:], oh[:, s, :],
                                         gpsw[:, s, 128:256],
                                         start=(s == 0), stop=(s == nsub - 1),
                                         skip_group_check=True)
                        nc.tensor.matmul(pw_d[:], oh[:, s, :],
                                         ones_sb[:],
                                         start=(s == 0), stop=(s == nsub - 1),
                                         skip_group_check=True)

                    if dbg and w == 0:
                        nc.sync.dma_start(dbg_gpr[:], gpr[:])
                        nc.sync.dma_start(dbg_gpsw[:], gpsw[:])
                        nc.sync.dma_start(dbg_z[:], z[:])
                        nc.sync.dma_start(dbg_h[:], h[:])
                        nc.sync.dma_start(dbg_a[:], a[:])
                        nc.sync.dma_start(dbg_ex[:], ex[:])
                        nc.sync.dma_start(dbg_oh[:], oh[:])
                        pncp = sp.tile([128, 128], DT.float32, tag="pncp")
                        nc.vector.tensor_copy(pncp[:], pw_n[:])
                        nc.sync.dma_start(dbg_pn[:], pncp[:])
                        pdcp = sp.tile([128, 1], DT.float32, tag="pdcp")
                        nc.vector.tensor_copy(pdcp[:], pw_d[:])
                        nc.sync.dma_start(dbg_pd[:], pdcp[:])
                    dn = sp.tile([128, 1], DT.float32, tag="dn")
                    nc.vector.tensor_scalar(dn[:], pw_d[:], 1e-30, None,
                                            ALU.add)
                    rn = sp.tile([128, 1], DT.float32, tag="rn")
                    nc.vector.reciprocal(rn[:], dn[:])
                    o1 = sp.tile([128, 128], DT.float32, tag="o1")
                    nc.vector.tensor_scalar(o1[:], pw_n[:], rn[:], None,
                                            ALU.mult)
                    o2 = sp.tile([128, 128], DT.float32, tag="o2")
                    nc.vector.scalar_tensor_tensor(o2[:], o1[:], ALPHA, o1[:],
                                                   ALU.mult, ALU.max)
                    rows = WIN if w < nw - 1 else p["last_rows"]
                    nc.sync.dma_start(out_d[w * WIN:w * WIN + rows, :],
                                      o2[0:rows, :])

    if compile:
        nc.compile()
    return nc


def _wrap16(arr):
    """[nw, capx] int16 slot-index array -> [128, nw*capx//16] wrapped and
    replicated across the 8 GPSIMD core partition groups."""
    nwn, capx = arr.shape
    w = arr.reshape(nwn, capx // 16, 16).transpose(2, 0, 1).reshape(
        16, nwn * (capx // 16))
    return np.tile(w, (8, 1)).copy()


def _prep(node, edge_index, W_lin, b_lin, W_att, b_att, a_vec, cfg=None):
    """Host-side sharding/preprocessing. Returns (in_maps, cap_lo, cap_hi, k)."""
    p = dict(DEFAULT_CFG)
    if cfg is not None:
        p.update(cfg)
    ncores, npc, nw = p["ncores"], p["npc"], p["nw"]
    node_pad_n, shard_pad_n, lo_base = p["node_pad"], p["shard_pad"], p["lo_base"]
    n_nodes = p["n_nodes"]
    recv = np.asarray(edge_index[0], dtype=np.int64)
    send = np.asarray(edge_index[1], dtype=np.int64)

    # ---- fold a_vec into the weights (sign trick) ----
    sa = np.asarray(a_vec[:, 0], dtype=np.float32)
    pos = sa >= 0
    perm = np.concatenate([np.where(pos)[0], np.where(~pos)[0]])
    k = int(pos.sum())
    cvec = np.where(pos, sa, ALPHA * sa).astype(np.float32)

    W_att = np.asarray(W_att, dtype=np.float32)
    w_r_hat = (W_att[:F_DIM, :] * cvec[None, :])[:, perm]
    w_s_hat = (W_att[F_DIM:, :] * cvec[None, :])[:, perm]
    b_hat = (np.asarray(b_att, np.float32) * cvec)[perm]

    # ---- edge bucketing ----
    cid = recv // npc
    rloc = recv - cid * npc
    wid = rloc >> 7
    grp = (send >= lo_base).astype(np.int64)
    key = (cid * nw + wid) * 2 + grp
    order = np.lexsort((send, rloc, key))
    ks, rs, ss = key[order], rloc[order], send[order]
    counts = np.bincount(key, minlength=ncores * nw * 2).reshape(
        ncores, nw, 2)
    starts = np.concatenate([[0], np.cumsum(counts.reshape(-1))])[:-1].reshape(
        ncores, nw, 2)

    cap_lo = int(np.ceil(counts[:, :, 0].max() / 128) * 128)
    cap_hi = int(np.ceil(max(counts[:, :, 1].max(), 1) / 128) * 128)
    cap = cap_lo + cap_hi
    nsub = cap // 128

    recvidx = np.zeros((ncores, nw, cap), np.int16)
    recvadj = np.full((ncores, nw, cap), -1000.0, np.float32)
    sendlo = np.zeros((ncores, nw, cap_lo), np.int16)
    sendhi = np.zeros((ncores, nw, cap_hi), np.int16)
    for c in range(ncores):
        for w in range(nw):
            nlo = counts[c, w, 0]
            b0 = starts[c, w, 0]
            recvidx[c, w, :nlo] = rs[b0:b0 + nlo]
            recvadj[c, w, :nlo] = rs[b0:b0 + nlo] - w * WIN
            sendlo[c, w, :nlo] = ss[b0:b0 + nlo]
            nhi = counts[c, w, 1]
            b1 = starts[c, w, 1]
            recvidx[c, w, cap_lo:cap_lo + nhi] = rs[b1:b1 + nhi]
            recvadj[c, w, cap_lo:cap_lo + nhi] = rs[b1:b1 + nhi] - w * WIN
            sendhi[c, w, :nhi] = ss[b1:b1 + nhi] - lo_base

    # ---- node arrays ----
    node = np.asarray(node, dtype=np.float32)
    node_pad = np.zeros((node_pad_n, F_DIM), BF16)
    node_pad[:n_nodes] = node.astype(BF16)

    iota = np.tile(np.arange(128, dtype=np.float32), (128, 1))
    bias_pair = np.zeros((128, 256), np.float32)
    bias_pair[:, 128:256] = np.asarray(b_lin, np.float32)[None, :]
    bias_r = np.tile(b_hat[None, :], (128, 1)).astype(np.float32)

    in_maps = []
    for c in range(ncores):
        shard = np.zeros((shard_pad_n, F_DIM), BF16)
        shard[:npc] = node[c * npc:(c + 1) * npc].astype(BF16)
        adj = recvadj[c].reshape(nw, nsub, 128).transpose(2, 0, 1).reshape(
            128, nw * nsub).copy()
        in_maps.append({
            "node_pad": node_pad,
            "node_shard": shard,
            "w_s": w_s_hat.astype(BF16),
            "w_lin": np.asarray(W_lin, np.float32).astype(BF16),
            "w_r": w_r_hat.astype(BF16),
            "bias_pair": bias_pair,
            "bias_r": bias_r,
            "iota": iota,
            "recvidx": _wrap16(recvidx[c]),
            "sendlo": _wrap16(sendlo[c]),
            "sendhi": _wrap16(sendhi[c]),
            "recvadj": adj,
        })
    return in_maps, cap_lo, cap_hi, k


def kernel(node, edge, edge_index, W_lin, b_lin, W_att, b_att, a_vec):
    in_maps, cap_lo, cap_hi, k = _prep(node, edge_index, W_lin, b_lin,
                                       W_att, b_att, a_vec)
    ckey = (cap_lo, cap_hi, k)
    if ckey not in _BUILD_CACHE:
        _BUILD_CACHE[ckey] = _build(cap_lo, cap_hi, k)
    nc = _BUILD_CACHE[ckey]
    res = bass_utils.run_bass_kernel_spmd(nc, in_maps,
                                          core_ids=list(range(NCORES)))
    out = np.concatenate([res.results[c]["out"] for c in range(NCORES)],
                         axis=0)
    return np.ascontiguousarray(out[:N_NODES]).astype(np.float32)


# revision 19
# speedup vs baseline: 140.1909x; 140.1909x over previous
"""GATv2 attention head (kgcnn AttentionHeadGATV2) on 8 Trainium2 NeuronCores.

Strategy (edge/graph parallelism, self-contained — no file reads):
  * Host: bucket edges by receiver node; core c owns receiver nodes
    [c*6250, (c+1)*6250). Within a core, edges are grouped into 49
    receiver "windows" of 128 nodes, each padded to a fixed capacity
    (split into send<32768 / send>=32768 halves so gather indices fit
    int16). All index/metadata arrays are precomputed on host.
  * Device precompute (replicated per core): psw = [P_s | w_n] where
    P_s = node @ (W_att[128:]*c)[:,perm],  w_n = node @ W_lin,
    and a per-core shard of P_r = node @ (W_att[:128]*c)[:,perm] + b_hat.
    The column scaling c and permutation fold a_vec into the leaky-relu
    (sign trick), so the attention logit becomes a plain row-sum.
    b_lin is folded in at the end (sum of attention weights is 1).
  * Device edge phase, per window: dma_gather P_r[recv], [P_s|w_n][send]
    (bf16, <=1024 idxs per gather); z = pr + ps; h = lrelu_{0.2/5}(z)
    via (z*a) max/min z on DVE (column split at k); a = rowsum(h) via
    per-subtile tensor_scalar accumulate; ex = exp(a) via tanh identity;
    one-hot (x ex) built by fused tensor_scalar(is_equal, mult);
    scatter-add via PE matmul into PSUM [128 nodes x 128] (+denominator
    column); out = lrelu_{0.2}(numer/denom + b_lin).
"""

import sys

sys.path.insert(0, "/opt/trn_rl_repo")

import numpy as np
import ml_dtypes

import concourse.bacc as bacc
import concourse.bass as bass
import concourse.mybir as mybir
import concourse.tile as tile
from concourse import bass_utils

DT = mybir.dt
ALU = mybir.AluOpType
ACTF = mybir.ActivationFunctionType
AXIS = mybir.AxisListType

BF16 = ml_dtypes.bfloat16

# Problem constants (hardcoded per the task contract).
N_NODES = 50000
N_EDGES = 800000
F_DIM = 128
UNITS = 128
ALPHA = 0.2
NCORES = 8
NPC = N_NODES // NCORES          # 6250 receiver nodes per core
WIN = 128                        # receiver-window size (PSUM partition dim)
NW = (NPC + WIN - 1) // WIN      # 49 windows per core
LAST_ROWS = NPC - (NW - 1) * WIN # 106 rows in the last window
ST_ROWS = 2048                   # precompute supertile rows
NODE_PAD = 51200                 # 25 * 2048
SHARD_PAD = 8192                 # 4 * 2048
LO_BASE = 32768                  # int16 gather-index split point
GMAX = 1024                      # max idxs per dma_gather (SWDGE ring limit)

_BUILD_CACHE = {}

DEFAULT_CFG = dict(
    n_nodes=N_NODES, n_edges=N_EDGES, ncores=NCORES, npc=NPC, nw=NW,
    last_rows=LAST_ROWS, node_pad=NODE_PAD, shard_pad=SHARD_PAD,
    lo_base=LO_BASE,
)


def _build(cap_lo, cap_hi, k_pos, cfg=None, compile=True, dbg=False):
    p = dict(DEFAULT_CFG)
    if cfg is not None:
        p.update(cfg)

    nsub_lo = cap_lo // 128
    nsub_hi = cap_hi // 128
    cap = cap_lo + cap_hi
    nsub = nsub_lo + nsub_hi
    cap16 = cap // 16
    lo16 = cap_lo // 16
    hi16 = cap_hi // 16
    nw = p["nw"]

    nc = bacc.Bacc("TRN2", target_bir_lowering=False, debug=False,
                   num_devices=p["ncores"], num_swdge_queues=4)

    node_pad_d = nc.dram_tensor("node_pad", [p["node_pad"], 128], DT.bfloat16,
                                kind="ExternalInput")
    node_shard_d = nc.dram_tensor("node_shard", [p["shard_pad"], 128],
                                  DT.bfloat16, kind="ExternalInput")
    w_s_d = nc.dram_tensor("w_s", [128, 128], DT.bfloat16, kind="ExternalInput")
    w_lin_d = nc.dram_tensor("w_lin", [128, 128], DT.bfloat16,
                             kind="ExternalInput")
    w_r_d = nc.dram_tensor("w_r", [128, 128], DT.bfloat16, kind="ExternalInput")
    blin_d = nc.dram_tensor("blin", [128, 128], DT.float32,
                            kind="ExternalInput")
    bias_r_d = nc.dram_tensor("bias_r", [128, 128], DT.float32,
                              kind="ExternalInput")
    iota_d = nc.dram_tensor("iota", [128, 128], DT.bfloat16,
                            kind="ExternalInput")
    recvidx_d = nc.dram_tensor("recvidx", [128, nw * cap16], DT.int16,
                               kind="ExternalInput")
    sendlo_d = nc.dram_tensor("sendlo", [128, nw * lo16], DT.int16,
                              kind="ExternalInput")
    sendhi_d = nc.dram_tensor("sendhi", [128, nw * hi16], DT.int16,
                              kind="ExternalInput")
    recvadj_d = nc.dram_tensor("recvadj", [128, nw * nsub], DT.float32,
                               kind="ExternalInput")
    out_d = nc.dram_tensor("out", [p["npc"], 128], DT.float32,
                           kind="ExternalOutput")
    if dbg:
        dbg_psw = nc.dram_tensor("dbg_psw", [p["node_pad"], 256], DT.bfloat16,
                                 kind="ExternalOutput")
        dbg_pr = nc.dram_tensor("dbg_pr", [p["shard_pad"], 128], DT.bfloat16,
                                kind="ExternalOutput")

    with tile.TileContext(nc) as tc:
        with (
            tc.tile_pool(name="consts", bufs=1) as cpool,
            tc.tile_pool(name="dram", bufs=1, space="DRAM") as dpool,
        ):
            psw_dram = dpool.tile([p["node_pad"], 256], DT.bfloat16)
            pr_dram = dpool.tile([p["shard_pad"], 128], DT.bfloat16)

            ws_sb = cpool.tile([128, 128], DT.bfloat16)
            nc.sync.dma_start(ws_sb[:], w_s_d[:])
            wlin_sb = cpool.tile([128, 128], DT.bfloat16)
            nc.sync.dma_start(wlin_sb[:], w_lin_d[:])
            wr_sb = cpool.tile([128, 128], DT.bfloat16)
            nc.sync.dma_start(wr_sb[:], w_r_d[:])
            blin_sb = cpool.tile([128, 128], DT.float32)
            nc.sync.dma_start(blin_sb[:], blin_d[:])
            biasr_sb = cpool.tile([128, 128], DT.float32)
            nc.sync.dma_start(biasr_sb[:], bias_r_d[:])
            iota_sb = cpool.tile([128, 128], DT.bfloat16)
            nc.sync.dma_start(iota_sb[:], iota_d[:])
            recvidx_sb = cpool.tile([128, nw * cap16], DT.int16)
            nc.sync.dma_start(recvidx_sb[:], recvidx_d[:])
            sendlo_sb = cpool.tile([128, nw * lo16], DT.int16)
            nc.sync.dma_start(sendlo_sb[:], sendlo_d[:])
            sendhi_sb = cpool.tile([128, nw * hi16], DT.int16)
            nc.sync.dma_start(sendhi_sb[:], sendhi_d[:])
            recvadj_sb = cpool.tile([128, nw * nsub], DT.float32)
            nc.sync.dma_start(recvadj_sb[:], recvadj_d[:])
            ones_sb = cpool.tile([128, 1], DT.bfloat16)
            nc.vector.memset(ones_sb[:], 1.0)

            # ---------------- precompute phase ----------------
            nsubt = ST_ROWS // 128
            with (
                tc.tile_pool(name="pcsb", bufs=3) as pc,
                tc.tile_pool(name="pcpsum", bufs=6, space="PSUM") as pcp,
            ):
                for st in range(p["node_pad"] // ST_ROWS):
                    ntile = pc.tile([128, ST_ROWS], DT.bfloat16, tag="nodeT")
                    nc.sync.dma_start(
                        ntile[:], node_pad_d[st * ST_ROWS:(st + 1) * ST_ROWS, :],
                        transpose=True)
                    rows = pc.tile([128, nsubt, 256], DT.bfloat16, tag="pswrow")
                    for j in range(nsubt):
                        ps = pcp.tile([128, 256], DT.float32, tag="pcps")
                        lhsT = ntile[:, j * 128:(j + 1) * 128]
                        nc.tensor.matmul(ps[:, 0:128], lhsT, ws_sb[:],
                                         start=True, stop=True)
                        nc.tensor.matmul(ps[:, 128:256], lhsT, wlin_sb[:],
                                         start=True, stop=True)
                        nc.scalar.copy(rows[:, j, :], ps[:])
                    r0 = st * ST_ROWS
                    nc.sync.dma_start(psw_dram[r0:r0 + ST_ROWS, :], rows[:])
                for st in range(p["shard_pad"] // ST_ROWS):
                    ntile = pc.tile([128, ST_ROWS], DT.bfloat16, tag="nodeT")
                    nc.sync.dma_start(
                        ntile[:],
                        node_shard_d[st * ST_ROWS:(st + 1) * ST_ROWS, :],
                        transpose=True)
                    rows2 = pc.tile([128, nsubt, 128], DT.bfloat16, tag="prrow")
                    for j in range(nsubt):
                        ps = pcp.tile([128, 256], DT.float32, tag="pcps")
                        lhsT = ntile[:, j * 128:(j + 1) * 128]
                        nc.tensor.matmul(ps[:, 0:128], lhsT, wr_sb[:],
                                         start=True, stop=True)
                        nc.vector.tensor_tensor(rows2[:, j, :], ps[:, 0:128],
                                                biasr_sb[:], ALU.add)
                    r0 = st * ST_ROWS
                    nc.sync.dma_start(pr_dram[r0:r0 + ST_ROWS, :], rows2[:])

            if dbg:
                nc.sync.dma_start(dbg_psw[:], psw_dram[:])
                nc.sync.dma_start(dbg_pr[:], pr_dram[:])

            # ---------------- edge phase ----------------
            with (
                tc.tile_pool(name="edge", bufs=3) as ep,
                tc.tile_pool(name="edge4", bufs=4) as ep4,
                tc.tile_pool(name="edge5", bufs=5) as ep5,
                tc.tile_pool(name="small", bufs=4) as sp,
                tc.tile_pool(name="epsum", bufs=4, space="PSUM") as pp,
            ):
                for w in range(nw):
                    gpr = ep4.tile([128, nsub, 128], DT.bfloat16, tag="gpr")
                    gmax = globals().get("GMAX_OVERRIDE", GMAX)
                    for g0 in range(0, cap, gmax):
                        gn = min(gmax, cap - g0)
                        nc.gpsimd.dma_gather(
                            gpr[:, g0 // 128:(g0 + gn) // 128, :], pr_dram[:],
                            recvidx_sb[:, w * cap16 + g0 // 16:
                                       w * cap16 + (g0 + gn) // 16],
                            gn, gn, 128, queue_num=0)
                    gpsw = ep5.tile([128, nsub, 256], DT.bfloat16, tag="gpsw")
                    for g0 in range(0, cap_lo, gmax):
                        gn = min(gmax, cap_lo - g0)
                        nc.gpsimd.dma_gather(
                            gpsw[:, g0 // 128:(g0 + gn) // 128, :], psw_dram[:],
                            sendlo_sb[:, w * lo16 + g0 // 16:
                                      w * lo16 + (g0 + gn) // 16],
                            gn, gn, 256, queue_num=0)
                    for g0 in range(0, cap_hi, gmax):
                        gn = min(gmax, cap_hi - g0)
                        nc.gpsimd.dma_gather(
                            gpsw[:, nsub_lo + g0 // 128:
                                 nsub_lo + (g0 + gn) // 128, :],
                            psw_dram[p["lo_base"]:, :],
                            sendhi_sb[:, w * hi16 + g0 // 16:
                                      w * hi16 + (g0 + gn) // 16],
                            gn, gn, 256, queue_num=0)

                    z = ep.tile([128, nsub, 128], DT.bfloat16, tag="z")
                    nc.vector.tensor_tensor(z[:], gpr[:], gpsw[:, :, 0:128],
                                            ALU.add)
                    h = ep.tile([128, nsub, 128], DT.bfloat16, tag="h")
                    if k_pos > 0:
                        nc.vector.scalar_tensor_tensor(
                            h[:, :, 0:k_pos], z[:, :, 0:k_pos], ALPHA,
                            z[:, :, 0:k_pos], ALU.mult, ALU.max)
                    if k_pos < 128:
                        nc.vector.scalar_tensor_tensor(
                            h[:, :, k_pos:128], z[:, :, k_pos:128], 1.0 / ALPHA,
                            z[:, :, k_pos:128], ALU.mult, ALU.min)
                    a = sp.tile([128, nsub], DT.float32, tag="a")
                    for s in range(nsub):
                        nc.vector.tensor_scalar(h[:, s, :], h[:, s, :], 1.0,
                                                None, ALU.mult, ALU.add,
                                                accum_out=a[:, s:s + 1])
                    # exp(a) = (1 + tanh(a/2)) / (1 - tanh(a/2))
                    t = sp.tile([128, nsub], DT.float32, tag="t")
                    nc.scalar.activation(t[:], a[:], ACTF.Tanh, scale=0.5)
                    d = sp.tile([128, nsub], DT.float32, tag="d")
                    nc.vector.tensor_scalar(d[:], t[:], -1.0, 1.0, ALU.mult,
                                            ALU.add)
                    rd = sp.tile([128, nsub], DT.float32, tag="rd")
                    nc.vector.reciprocal(rd[:], d[:])
                    ex = sp.tile([128, nsub], DT.float32, tag="ex")
                    nc.vector.scalar_tensor_tensor(ex[:], t[:], 1.0, rd[:],
                                                   ALU.add, ALU.mult)

                    oh = ep4.tile([128, nsub, 128], DT.bfloat16, tag="oh")
                    for s in range(nsub):
                        nc.vector.tensor_scalar(
                            oh[:, s, :], iota_sb[:],
                            recvadj_sb[:, w * nsub + s:w * nsub + s + 1],
                            ex[:, s:s + 1], ALU.is_equal, ALU.mult)

                    pw_n = pp.tile([128, 128], DT.float32, tag="pwn")
                    pw_d = pp.tile([128, 1], DT.float32, tag="pwd")
                    for s in range(nsub):
                        nc.tensor.matmul(pw_n[:], oh[:, s, :],
                                         gpsw[:, s, 128:256],
                                         start=(s == 0), stop=(s == nsub - 1),
                                         skip_group_check=True)
                        nc.tensor.matmul(pw_d[:], oh[:, s, :],
                                         ones_sb[:],
                                         start=(s == 0), stop=(s == nsub - 1),
                                         skip_group_check=True)

                    dn = sp.tile([128, 1], DT.float32, tag="dn")
                    nc.vector.tensor_scalar(dn[:], pw_d[:], 1e-30, None,
                                            ALU.add)
                    rn = sp.tile([128, 1], DT.float32, tag="rn")
                    nc.vector.reciprocal(rn[:], dn[:])
                    o1 = sp.tile([128, 128], DT.float32, tag="o1")
                    nc.vector.scalar_tensor_tensor(o1[:], pw_n[:], rn[:],
                                                   blin_sb[:], ALU.mult,
                                                   ALU.add)
                    o2 = sp.tile([128, 128], DT.float32, tag="o2")
                    nc.vector.scalar_tensor_tensor(o2[:], o1[:], ALPHA, o1[:],
                                                   ALU.mult, ALU.max)
                    rows = WIN if w < nw - 1 else p["last_rows"]
                    nc.sync.dma_start(out_d[w * WIN:w * WIN + rows, :],
                                      o2[0:rows, :])

    if compile:
        nc.compile()
    return nc


def _store_perm(i):
    """Map a logical row index to its stored row in psw/pr DRAM.

    The precompute writes SBUF tiles [128, J, 256] with one dma_start per
    ST_ROWS block; the DMA lays out (partition p, subtile j) at block row
    p*J + j, while logical row r = j*128 + p. Gather indices must follow."""
    J = ST_ROWS // 128
    b = i // ST_ROWS
    r = i % ST_ROWS
    return b * ST_ROWS + (r % 128) * J + (r // 128)


def _wrap16(arr):
    """[nw, capx] int16 slot-index array -> [128, nw*capx//16] wrapped and
    replicated across the 8 GPSIMD core partition groups."""
    nwn, capx = arr.shape
    w = arr.reshape(nwn, capx // 16, 16).transpose(2, 0, 1).reshape(
        16, nwn * (capx // 16))
    return np.tile(w, (8, 1)).copy()


def _prep(node, edge_index, W_lin, b_lin, W_att, b_att, a_vec, cfg=None):
    """Host-side sharding/preprocessing. Returns (in_maps, cap_lo, cap_hi, k)."""
    p = dict(DEFAULT_CFG)
    if cfg is not None:
        p.update(cfg)
    ncores, npc, nw = p["ncores"], p["npc"], p["nw"]
    node_pad_n, shard_pad_n, lo_base = (p["node_pad"], p["shard_pad"],
                                        p["lo_base"])
    n_nodes = p["n_nodes"]

    recv = np.asarray(edge_index[0], dtype=np.int64)
    send = np.asarray(edge_index[1], dtype=np.int64)

    # ---- fold a_vec into the weights (sign trick) ----
    sa = np.asarray(a_vec[:, 0], dtype=np.float32)
    pos = sa >= 0
    perm = np.concatenate([np.where(pos)[0], np.where(~pos)[0]])
    k = int(pos.sum())
    cvec = np.where(pos, sa, ALPHA * sa).astype(np.float32)

    W_att = np.asarray(W_att, dtype=np.float32)
    w_r_hat = (W_att[:F_DIM, :] * cvec[None, :])[:, perm]
    w_s_hat = (W_att[F_DIM:, :] * cvec[None, :])[:, perm]
    b_hat = (np.asarray(b_att, np.float32) * cvec)[perm]

    # ---- edge bucketing ----
    cid = recv // npc
    rloc = recv - cid * npc
    wid = rloc >> 7
    grp = (send >= lo_base).astype(np.int64)
    key = (cid * nw + wid) * 2 + grp
    order = np.lexsort((send, rloc, key))
    ks, rs, ss = key[order], rloc[order], send[order]
    counts = np.bincount(key, minlength=ncores * nw * 2).reshape(ncores, nw, 2)
    starts = np.concatenate([[0], np.cumsum(counts.reshape(-1))])[:-1].reshape(
        ncores, nw, 2)

    cap_lo = int(np.ceil(counts[:, :, 0].max() / 128) * 128)
    cap_hi = int(np.ceil(max(counts[:, :, 1].max(), 1) / 128) * 128)
    cap = cap_lo + cap_hi
    nsub = cap // 128

    recvidx = np.zeros((ncores, nw, cap), np.int16)
    recvadj = np.full((ncores, nw, cap), -1000.0, np.float32)
    sendlo = np.zeros((ncores, nw, cap_lo), np.int16)
    sendhi = np.zeros((ncores, nw, cap_hi), np.int16)
    for c in range(ncores):
        for w in range(nw):
            nlo = counts[c, w, 0]
            b0 = starts[c, w, 0]
            recvidx[c, w, :nlo] = _store_perm(rs[b0:b0 + nlo])
            recvadj[c, w, :nlo] = rs[b0:b0 + nlo] - w * WIN
            sendlo[c, w, :nlo] = _store_perm(ss[b0:b0 + nlo])
            nhi = counts[c, w, 1]
            b1 = starts[c, w, 1]
            recvidx[c, w, cap_lo:cap_lo + nhi] = _store_perm(rs[b1:b1 + nhi])
            recvadj[c, w, cap_lo:cap_lo + nhi] = rs[b1:b1 + nhi] - w * WIN
            sendhi[c, w, :nhi] = _store_perm(ss[b1:b1 + nhi]) - lo_base

    # ---- node arrays ----
    node = np.asarray(node, dtype=np.float32)
    node_pad = np.zeros((node_pad_n, F_DIM), BF16)
    node_pad[:n_nodes] = node.astype(BF16)

    iota = np.tile(np.arange(128, dtype=np.float32), (128, 1)).astype(BF16)
    blin = np.tile(np.asarray(b_lin, np.float32)[None, :], (128, 1))
    bias_r = np.tile(b_hat[None, :], (128, 1)).astype(np.float32)

    in_maps = []
    for c in range(ncores):
        shard = np.zeros((shard_pad_n, F_DIM), BF16)
        shard[:npc] = node[c * npc:(c + 1) * npc].astype(BF16)
        adj = recvadj[c].reshape(nw, nsub, 128).transpose(2, 0, 1).reshape(
            128, nw * nsub).copy()
        in_maps.append({
            "node_pad": node_pad,
            "node_shard": shard,
            "w_s": w_s_hat.astype(BF16),
            "w_lin": np.asarray(W_lin, np.float32).astype(BF16),
            "w_r": w_r_hat.astype(BF16),
            "blin": blin,
            "bias_r": bias_r,
            "iota": iota,
            "recvidx": _wrap16(recvidx[c]),
            "sendlo": _wrap16(sendlo[c]),
            "sendhi": _wrap16(sendhi[c]),
            "recvadj": adj,
        })
    return in_maps, cap_lo, cap_hi, k


def kernel(node, edge, edge_index, W_lin, b_lin, W_att, b_att, a_vec):
    in_maps, cap_lo, cap_hi, k = _prep(node, edge_index, W_lin, b_lin,
                                       W_att, b_att, a_vec)
    ckey = (cap_lo, cap_hi, k)
    if ckey not in _BUILD_CACHE:
        _BUILD_CACHE[ckey] = _build(cap_lo, cap_hi, k)
    nc = _BUILD_CACHE[ckey]
    res = bass_utils.run_bass_kernel_spmd(nc, in_maps,
                                          core_ids=list(range(NCORES)))
    out = np.concatenate([res.results[c]["out"] for c in range(NCORES)],
                         axis=0)
    return np.ascontiguousarray(out[:N_NODES]).astype(np.float32)
